# revision 1
# baseline (speedup 1.0000x reference)
"""Trainium2 Bass kernel for nn_CoordinateDecoder.

Computation (see reference): posenc(coords) ++ trilinear-pyramid-sampled
features -> 5-layer MLP (gelu-tanh approx, skip concat at depth 2, tanh out).

Strategy:
  - Data-parallel over B: core b handles batch image b (coords/weights shared).
  - Bilinear pyramid sampling is done ON THE TENSOR ENGINE: samples are
    host-sorted by their continuous y coordinate, so for every pyramid level
    the samples that read a given 2-row band of the grid are contiguous.
    Sampling then becomes, per y-bucket, a matmul
        out[256ch, n_run] = RP[bucket][128 grid-cells, 256ch]^T @ S[128, n_run]
    where S holds the 4 bilinear weights per sample (built dense on host,
    shipped bf16).  This produces features directly in feature-major layout
    (channels on partitions), which is what the MLP matmuls need.
  - MLP runs in bf16 (fp32 PSUM accumulation), weights stationary, N=512
    moving tiles.  Gelu (tanh approx) + bias fused on the scalar engine.
  - Host does only O(N) / O(grid) prep: pyramid resize (134 MMAC), posenc,
    bilinear index/weight computation, argsort, packing.  All heavy compute
    (80 GMAC of matmul) is on device.
"""

import numpy as np
import ml_dtypes

BF16 = ml_dtypes.bfloat16

B, H, W, C = 8, 64, 64, 256
N = 16384
NUM_FREQS = 10
MLP_WIDTH = 256
IN_DIM = 2 + 4 * NUM_FREQS + 3 * C  # 810

NSUP = 8            # column supers
SUP = N // NSUP     # 2048
NCH = 4             # 512-chunks per super
CH = 512

LEVEL_SIZES = [64, 32, 16]
# per-level k-layout of the RP (row-pair) stationary tensors:
#   L0: bucket g in [0,63): partitions r*64+x  = grid rows (g, g+1)
#   L1: bucket b in [0,11): partitions r*32+x  = grid rows (3b .. 3b+3)
#   L2: quad   q in [0,4):  partitions 32*rb + dy*16 + x = rows (4q+rb, 4q+rb+1)
N_BUCKETS = [63, 11, 4]


def _resize_matrix(out_size: int, in_size: int) -> np.ndarray:
    """Row-resize operator of jax.image.resize(..., 'bilinear') (antialias).
    Returns M [out_size, in_size] with resized = M @ x."""
    scale = out_size / in_size
    inv_scale = 1.0 / scale
    kernel_scale = max(inv_scale, 1.0)
    sample_f = (np.arange(out_size, dtype=np.float64) + 0.5) * inv_scale - 0.5
    x = np.abs(sample_f[None, :] - np.arange(in_size, dtype=np.float64)[:, None])
    x = x / kernel_scale
    w = np.where(x < 1.0, 1.0 - x, 0.0)
    total = w.sum(axis=0, keepdims=True)
    w = np.where(
        np.abs(total) > 1000.0 * np.finfo(np.float32).eps,
        w / np.where(total != 0.0, total, 1.0),
        0.0,
    )
    w = np.where(
        ((sample_f >= -0.5) & (sample_f <= in_size - 0.5))[None, :], w, 0.0
    )
    return w.T.astype(np.float32)  # [out, in]


def _posenc_t(coords: np.ndarray) -> np.ndarray:
    """Transposed positional encoding [42, n] fp32, matching reference order."""
    freqs = (2.0 ** np.arange(NUM_FREQS, dtype=np.float32)) * np.float32(np.pi)
    parts = [coords.T.astype(np.float32)]
    for f in freqs:
        parts.append(np.sin(coords.T * f).astype(np.float32))
        parts.append(np.cos(coords.T * f).astype(np.float32))
    return np.concatenate(parts, axis=0)  # [42, n]


def _bilinear(c01: np.ndarray, size: int):
    """c01 [n] in [0,1] -> (i0, frac) fp32 like the reference's fp32 math."""
    cr = (c01 * np.float32(size - 1)).astype(np.float32)
    i0 = np.floor(cr).astype(np.int64)
    i0 = np.clip(i0, 0, size - 2)
    f = cr - i0.astype(np.float32)
    return i0, f.astype(np.float32)


def _host_prep(feature_grid, coords, w0, b0, w1, b1, w2, b2, w3, b3, w_out, b_out):
    """All host-side packing. Returns (shared_map, per_core_maps, perm, runs)."""
    fg = np.asarray(feature_grid, dtype=np.float32)
    coords = np.asarray(coords, dtype=np.float32)

    # ---- sort samples by continuous y so every level's y-buckets are runs ----
    c01 = (coords + np.float32(1.0)) / np.float32(2.0)  # [N,2] (y, x)
    perm = np.argsort(c01[:, 0], kind="stable")
    c01s = c01[perm]
    coords_s = coords[perm]

    # ---- per-level bilinear indices / weights / buckets -----------------------
    y0, fy, x0, fx, buckets = [], [], [], [], []
    for li, S in enumerate(LEVEL_SIZES):
        yi, fyi = _bilinear(c01s[:, 0], S)
        xi, fxi = _bilinear(c01s[:, 1], S)
        y0.append(yi); fy.append(fyi); x0.append(xi); fx.append(fxi)
        if li == 0:
            buckets.append(yi.copy())
        elif li == 1:
            buckets.append(yi // 3)
        else:
            buckets.append(yi // 4)

    # ---- dense S^T matrices [128, N] bf16 ------------------------------------
    s_t = []
    for li in range(3):
        Sm = np.zeros((N, 128), np.float32)
        wtl = (1 - fy[li]) * (1 - fx[li])
        wtr = (1 - fy[li]) * fx[li]
        wbl = fy[li] * (1 - fx[li])
        wbr = fy[li] * fx[li]
        j = np.arange(N)
        if li == 0:
            ktop = x0[li]
            kbot = 64 + x0[li]
        elif li == 1:
            dy_loc = y0[li] - 3 * buckets[li]
            ktop = dy_loc * 32 + x0[li]
            kbot = (dy_loc + 1) * 32 + x0[li]
        else:
            rb = y0[li] - 4 * buckets[li]
            ktop = rb * 32 + x0[li]
            kbot = rb * 32 + 16 + x0[li]
        Sm[j, ktop] = wtl
        Sm[j, ktop + 1] = wtr
        Sm[j, kbot] = wbl
        Sm[j, kbot + 1] = wbr
        s_t.append(np.ascontiguousarray(Sm.T).astype(BF16))

    # ---- bucket runs, split at CH boundaries ---------------------------------
    runs = []  # runs[level][chunk] = list of (bucket, off_in_chunk, length)
    for li in range(3):
        bk = buckets[li]
        per_chunk = [[] for _ in range(N // CH)]
        start = 0
        while start < N:
            g = bk[start]
            end = start
            while end < N and bk[end] == g:
                end += 1
            # split [start, end) at chunk boundaries
            p = start
            while p < end:
                ci = p // CH
                q = min(end, (ci + 1) * CH)
                per_chunk[ci].append((int(g), p - ci * CH, q - p))
                p = q
            start = end
        runs.append(per_chunk)

    # ---- pyramid + row-pair (RP) tensors per core ----------------------------
    R1 = _resize_matrix(32, 64)
    R2 = _resize_matrix(16, 64)
    g1 = np.einsum("ph,qw,bhwc->bpqc", R1, R1, fg, optimize=True)
    g2 = np.einsum("ph,qw,bhwc->bpqc", R2, R2, fg, optimize=True)

    def rp_tensors(g0b, g1b, g2b):
        # L0: [128, 63*256]: bucket g -> rows (g, g+1), partitions r*64+x
        rp0 = np.zeros((128, 63 * 256), np.float32)
        for g in range(63):
            blk = g0b[g:g + 2]                      # [2, 64, 256]
            rp0[:, g * 256:(g + 1) * 256] = blk.reshape(128, 256)
        # L1: [128, 11*256]: bucket b -> rows 3b..3b+3 (pad past row 31)
        rp1 = np.zeros((128, 11 * 256), np.float32)
        for b in range(11):
            rows = g1b[3 * b:3 * b + 4]             # up to [4, 32, 256]
            blk = np.zeros((4, 32, 256), np.float32)
            blk[:rows.shape[0]] = rows
            rp1[:, b * 256:(b + 1) * 256] = blk.reshape(128, 256)
        # L2: [128, 4*256]: quad q, block rb -> rows (4q+rb, 4q+rb+1)
        rp2 = np.zeros((128, 4 * 256), np.float32)
        for q in range(4):
            blk = np.zeros((4, 2, 16, 256), np.float32)
            for rb in range(4):
                rows = g2b[4 * q + rb:4 * q + rb + 2]
                blk[rb, :rows.shape[0]] = rows
            rp2[:, q * 256:(q + 1) * 256] = blk.reshape(128, 256)
        return rp0.astype(BF16), rp1.astype(BF16), rp2.astype(BF16)

    per_core = []
    for b in range(B):
        rp0, rp1, rp2 = rp_tensors(fg[b], g1[b], g2[b])
        per_core.append({"rp0": rp0, "rp1": rp1, "rp2": rp2})

    # ---- posenc (padded to a full 128-row k-tile) ----------------------------
    enc = np.zeros((128, N), np.float32)
    enc[:42] = _posenc_t(coords_s)
    enc = enc.astype(BF16)

    # ---- weights: reorder rows into the device k-layout, pack [128, kt*M] ----
    w0 = np.asarray(w0, np.float32); w1 = np.asarray(w1, np.float32)
    w2 = np.asarray(w2, np.float32); w3 = np.asarray(w3, np.float32)
    w_out = np.asarray(w_out, np.float32)

    def pack(wd):  # [Ktot, M] -> [128, (Ktot/128) * M], k-tile major
        K, M = wd.shape
        assert K % 128 == 0
        return np.ascontiguousarray(
            wd.reshape(K // 128, 128, M).transpose(1, 0, 2).reshape(128, -1)
        )

    w0d = np.zeros((896, 256), np.float32)
    w0d[0:42] = w0[0:42]          # enc
    w0d[128:384] = w0[42:298]     # L0
    w0d[384:640] = w0[298:554]    # L1
    w0d[640:896] = w0[554:810]    # L2
    w3d = np.zeros((1152, 256), np.float32)
    w3d[0:256] = w3[0:256]        # h
    w3d[256:298] = w3[256:298]    # enc
    w3d[384:640] = w3[298:554]    # L0
    w3d[640:896] = w3[554:810]    # L1
    w3d[896:1152] = w3[810:1066]  # L2
    woutd = np.zeros((256, 3), np.float32)
    woutd[:] = w_out

    shared = {
        "s0t": s_t[0], "s1t": s_t[1], "s2t": s_t[2], "enc": enc,
        "w0": pack(w0d).astype(BF16), "w1": pack(w1).astype(BF16),
        "w2": pack(w2).astype(BF16), "w3": pack(w3d).astype(BF16),
        "wout": pack(woutd).astype(BF16),
        "b0": np.asarray(b0, np.float32).reshape(2, 128).T.copy(),
        "b1": np.asarray(b1, np.float32).reshape(2, 128).T.copy(),
        "b2": np.asarray(b2, np.float32).reshape(2, 128).T.copy(),
        "b3": np.asarray(b3, np.float32).reshape(2, 128).T.copy(),
        "bout": np.asarray(b_out, np.float32).reshape(3, 1).copy(),
    }
    return shared, per_core, perm, runs


_DRAM_SPECS = [
    # name, shape, np dtype
    ("rp0", (128, 63 * 256), BF16),
    ("rp1", (128, 11 * 256), BF16),
    ("rp2", (128, 4 * 256), BF16),
    ("s0t", (128, N), BF16),
    ("s1t", (128, N), BF16),
    ("s2t", (128, N), BF16),
    ("enc", (128, N), BF16),
    ("w0", (128, 7 * 256), BF16),
    ("w1", (128, 2 * 256), BF16),
    ("w2", (128, 2 * 256), BF16),
    ("w3", (128, 9 * 256), BF16),
    ("wout", (128, 2 * 3), BF16),
    ("b0", (128, 2), np.float32),
    ("b1", (128, 2), np.float32),
    ("b2", (128, 2), np.float32),
    ("b3", (128, 2), np.float32),
    ("bout", (3, 1), np.float32),
]


def _build_nc(runs):
    """Build the Bacc program (shared by all cores; per-core data differs)."""
    from contextlib import ExitStack

    import concourse.bacc as bacc
    import concourse.mybir as mybir
    import concourse.tile as tile

    bf16 = mybir.dt.bfloat16
    f32 = mybir.dt.float32
    GELU = mybir.ActivationFunctionType.Gelu_apprx_tanh
    TANH = mybir.ActivationFunctionType.Tanh

    nc = bacc.Bacc("TRN2", debug=False, target_bir_lowering=False)

    dram = {}
    for name, shape, npdt in _DRAM_SPECS:
        dram[name] = nc.dram_tensor(
            name, list(shape), mybir.dt.from_np(np.dtype(npdt)), kind="ExternalInput"
        )
    out_dram = nc.dram_tensor("out_t", [3, N], f32, kind="ExternalOutput")

    with tile.TileContext(nc) as tc, ExitStack() as ctx:
        const = ctx.enter_context(tc.tile_pool(name="const", bufs=1))
        spool = ctx.enter_context(tc.tile_pool(name="stream", bufs=2))
        xtpool = ctx.enter_context(tc.tile_pool(name="xt", bufs=2))
        hpool = ctx.enter_context(tc.tile_pool(name="h", bufs=5))
        opool = ctx.enter_context(tc.tile_pool(name="osb", bufs=2))
        ps_samp = ctx.enter_context(tc.tile_pool(name="ps_samp", bufs=3, space="PSUM"))
        ps_mlp = ctx.enter_context(tc.tile_pool(name="ps_mlp", bufs=4, space="PSUM"))
        ps_out = ctx.enter_context(tc.tile_pool(name="ps_out", bufs=1, space="PSUM"))

        # ---- static tensors ---------------------------------------------------
        st = {}
        # load order matters: small rp tensors first so sampling (L2, L1)
        # can start while the 4MB rp0 is still in flight; rp0 is split into
        # 4 independent quarter-loads so low buckets unblock early.
        order = ["rp2", "rp1", "rp0",
                 "w0", "w1", "w2", "w3", "wout", "b0", "b1", "b2", "b3", "bout"]
        specs = {n: (s, d) for n, s, d in _DRAM_SPECS}
        for name in order:
            if name not in specs:
                continue
            shape, npdt = specs[name]
            t = const.tile(list(shape), mybir.dt.from_np(np.dtype(npdt)), tag=name)
            if name == "rp0":
                q = shape[1] // 4
                for i in range(4):
                    nc.sync.dma_start(t[:, i * q:(i + 1) * q],
                                      dram[name][:, i * q:(i + 1) * q])
            else:
                nc.sync.dma_start(t[:, :], dram[name][:, :])
            st[name] = t

        rp = [st["rp0"], st["rp1"], st["rp2"]]
        wmlp = [st["w0"], st["w1"], st["w2"], st["w3"]]
        bmlp = [st["b0"], st["b1"], st["b2"], st["b3"]]
        KT = [7, 2, 2, 9]

        for s in range(NSUP):
            lo = s * SUP
            sl = slice(lo, lo + SUP)
            s_tiles = []
            for nm in ("s0t", "s1t", "s2t"):
                t = spool.tile([128, SUP], bf16, tag=nm)
                nc.sync.dma_start(t[:, :], dram[nm][:, sl])
                s_tiles.append(t)

            # X^T for this super: k-tiles [enc, L0a, L0b, L1a, L1b, L2a, L2b]
            xt = xtpool.tile([128, 7 * SUP], bf16, tag="xt")
            nc.sync.dma_start(xt[:, 0:SUP], dram["enc"][:, sl])

            # ---- sampling: per (m-tile, level, chunk) -------------------------
            for m in range(2):
                for li in range(3):
                    for ch in range(NCH):
                        p = ps_samp.tile([128, CH], f32, tag="ps_samp")
                        for (g, off, ln) in runs[li][s * NCH + ch]:
                            nc.tensor.matmul(
                                p[:, off:off + ln],
                                rp[li][:, g * 256 + m * 128: g * 256 + m * 128 + 128],
                                s_tiles[li][:, ch * CH + off: ch * CH + off + ln],
                                start=True, stop=True,
                            )
                        dst = (1 + 2 * li + m) * SUP + ch * CH
                        nc.vector.tensor_copy(xt[:, dst:dst + CH], p[:, :])

            # ---- MLP ---------------------------------------------------------
            def dense(layer, rhs_fn):
                h = hpool.tile([128, 2 * SUP], bf16, tag="h")
                for m in range(2):
                    pss = [ps_mlp.tile([128, CH], f32, tag="ps_mlp", name=f"ps_mlp_{layer}_{m}_{i}")
                           for i in range(NCH)]
                    for kt in range(KT[layer]):
                        lhsT = wmlp[layer][:, kt * 256 + m * 128:
                                           kt * 256 + m * 128 + 128]
                        for ns in range(NCH):
                            nc.tensor.matmul(
                                pss[ns][:, :], lhsT, rhs_fn(kt, ns),
                                start=(kt == 0), stop=(kt == KT[layer] - 1),
                            )
                    for ns in range(NCH):
                        nc.scalar.activation(
                            h[:, m * SUP + ns * CH: m * SUP + ns * CH + CH],
                            pss[ns][:, :], GELU, bias=bmlp[layer][:, m:m + 1],
                        )
                return h

            h0 = dense(0, lambda kt, ns: xt[:, kt * SUP + ns * CH: kt * SUP + ns * CH + CH])
            h1 = dense(1, lambda kt, ns: h0[:, kt * SUP + ns * CH: kt * SUP + ns * CH + CH])
            h2 = dense(2, lambda kt, ns: h1[:, kt * SUP + ns * CH: kt * SUP + ns * CH + CH])

            def rhs3(kt, ns):
                src = h2 if kt < 2 else xt
                k = kt if kt < 2 else kt - 2
                return src[:, k * SUP + ns * CH: k * SUP + ns * CH + CH]

            h3 = dense(3, rhs3)

            # ---- output layer -------------------------------------------------
            osb = opool.tile([3, SUP], f32, tag="osb")
            for ns in range(NCH):
                po = ps_out.tile([128, CH], f32, tag="ps_out")
                for kt in range(2):
                    nc.tensor.matmul(
                        po[:3, :],
                        st["wout"][:, kt * 3:(kt + 1) * 3],
                        h3[:, kt * SUP + ns * CH: kt * SUP + ns * CH + CH],
                        start=(kt == 0), stop=(kt == 1),
                    )
                nc.scalar.activation(
                    osb[:, ns * CH:(ns + 1) * CH], po[:3, :], TANH,
                    bias=st["bout"][:, 0:1],
                )
            nc.sync.dma_start(out_dram[:, sl], osb[:, :])

    nc.compile()
    return nc


def kernel(feature_grid, coords, w0, b0, w1, b1, w2, b2, w3, b3, w_out, b_out,
           _run_opts=None):
    from concourse.bass_utils import run_bass_kernel_spmd

    shared, per_core, perm, runs = _host_prep(
        feature_grid, coords, w0, b0, w1, b1, w2, b2, w3, b3, w_out, b_out)

    nc = _build_nc(runs)

    in_maps = []
    for b in range(B):
        m = dict(shared)
        m.update(per_core[b])
        in_maps.append(m)

    res = run_bass_kernel_spmd(
        nc, in_maps, core_ids=list(range(B)), **(_run_opts or {})
    )

    out = np.empty((B, N, 3), np.float32)
    inv = perm  # out_sorted column j corresponds to original sample perm[j]
    for b in range(B):
        out[b, inv, :] = res.results[b]["out_t"].T
    if _run_opts is not None:
        kernel._last_result = res  # for test harness introspection
    return out



# revision 2
# speedup vs baseline: 1.3877x; 1.3877x over previous
"""Trainium2 Bass kernel for nn_CoordinateDecoder.

Computation (see reference): posenc(coords) ++ bilinear-pyramid-sampled
features -> 5-layer MLP (gelu-tanh approx, skip concat at depth 2, tanh out).

Strategy:
  - Data-parallel over B: core b handles batch image b (coords/weights shared).
  - KEY TRICK: bilinear sampling is linear, so the sampled features' MLP
    contribution folds into the grids on the host:
        feats @ W0_lvl = S_lvl^T (G_lvl @ W0_lvl) = S_lvl^T A_lvl .
    The on-device sampling matmuls therefore emit layer-0 (and layer-3 skip)
    PRE-ACTIVATIONS directly, accumulated in PSUM across pyramid levels.
    This removes the layer-0 GEMM (7 k-tiles) and 6 of 9 k-tiles of layer 3.
  - Samples are host-sorted by continuous y.  Per 512-sample chunk:
      L0 (64x64 grid): row-pair bucket runs (matmul per run, k=128 cells).
      L1 (32x32 grid): one full-width matmul, k = 4-grid-row window (4x32).
      L2 (16x16 grid): one full-width matmul, k = 4-row window (4x16=64 cells)
        + the 42 posenc dims folded into spare k-rows 64:106 (enc weight rows
        live in the L2 stationary block; enc values live in s2t rows 64:106).
    The full-width L2 matmul is the PSUM bank's start=True anchor and the
    full-width L1 matmul its stop=True anchor; partial L0-run matmuls are
    ordered between them by byte-range WAW deps (start=True pending-zeroes the
    whole bank; start=False overwrites on first touch, accumulates after).
  - MLP in bf16 (fp32 PSUM), gelu fused on ScalarE as one [128,2048] call per
    (layer, m-half) spanning 4 PSUM banks to amortize ACT overhead.
  - Host does O(N)/O(grid) prep: pyramid resize, A = G @ W folds (~11 GFLOP
    BLAS), posenc, bilinear indices, argsort, packing.
"""

import numpy as np
import ml_dtypes

BF16 = ml_dtypes.bfloat16

B, H, W, C = 8, 64, 64, 256
N = 16384
NUM_FREQS = 10
MLP_WIDTH = 256

NSUP = 8            # column supers
SUP = N // NSUP     # 2048
CH = 512            # psum-bank chunk
NCH_S = SUP // CH   # 4 chunks per super
NCHUNK = N // CH    # 32

LEVEL_SIZES = [64, 32, 16]


def _resize_matrix(out_size: int, in_size: int) -> np.ndarray:
    """Row-resize operator of jax.image.resize(..., 'bilinear') (antialias).
    Returns M [out, in] with resized = M @ x."""
    scale = out_size / in_size
    inv_scale = 1.0 / scale
    kernel_scale = max(inv_scale, 1.0)
    sample_f = (np.arange(out_size, dtype=np.float64) + 0.5) * inv_scale - 0.5
    x = np.abs(sample_f[None, :] - np.arange(in_size, dtype=np.float64)[:, None])
    x = x / kernel_scale
    w = np.where(x < 1.0, 1.0 - x, 0.0)
    total = w.sum(axis=0, keepdims=True)
    w = np.where(
        np.abs(total) > 1000.0 * np.finfo(np.float32).eps,
        w / np.where(total != 0.0, total, 1.0),
        0.0,
    )
    w = np.where(
        ((sample_f >= -0.5) & (sample_f <= in_size - 0.5))[None, :], w, 0.0
    )
    return w.T.astype(np.float32)  # [out, in]


def _posenc_t(coords: np.ndarray) -> np.ndarray:
    """Transposed positional encoding [42, n] fp32, matching reference order."""
    freqs = (2.0 ** np.arange(NUM_FREQS, dtype=np.float32)) * np.float32(np.pi)
    parts = [coords.T.astype(np.float32)]
    for f in freqs:
        parts.append(np.sin(coords.T * f).astype(np.float32))
        parts.append(np.cos(coords.T * f).astype(np.float32))
    return np.concatenate(parts, axis=0)  # [42, n]


def _bilinear(c01: np.ndarray, size: int):
    """c01 [n] in [0,1] -> (i0, frac) fp32 like the reference's fp32 math."""
    cr = (c01 * np.float32(size - 1)).astype(np.float32)
    i0 = np.floor(cr).astype(np.int64)
    i0 = np.clip(i0, 0, size - 2)
    f = cr - i0.astype(np.float32)
    return i0, f.astype(np.float32)


def _windows(ys: np.ndarray):
    """Per 512-chunk segments [(off, ln, wbase)] s.t. all y0 in a segment fit
    rows [wbase, wbase+4) (y0 and y0+1 both inside).  ys nondecreasing."""
    segs = []
    for c in range(NCHUNK):
        yc = ys[c * CH:(c + 1) * CH]
        lst = []
        p = 0
        while p < CH:
            wb = int(yc[p])
            q = p
            while q < CH and yc[q] <= wb + 2:
                q += 1
            lst.append((p, q - p, wb))
            p = q
        segs.append(lst)
    return segs


def _host_prep(feature_grid, coords, w0, b0, w1, b1, w2, b2, w3, b3, w_out, b_out):
    fg = np.asarray(feature_grid, dtype=np.float32)
    coords = np.asarray(coords, dtype=np.float32)
    w0 = np.asarray(w0, np.float32); w1 = np.asarray(w1, np.float32)
    w2 = np.asarray(w2, np.float32); w3 = np.asarray(w3, np.float32)
    w_out = np.asarray(w_out, np.float32)

    # ---- sort samples by continuous y --------------------------------------
    c01 = (coords + np.float32(1.0)) / np.float32(2.0)  # [N,2] (y, x)
    perm = np.argsort(c01[:, 0], kind="stable")
    c01s = c01[perm]
    coords_s = coords[perm]

    y0, fy, x0, fx = [], [], [], []
    for S in LEVEL_SIZES:
        yi, fyi = _bilinear(c01s[:, 0], S)
        xi, fxi = _bilinear(c01s[:, 1], S)
        y0.append(yi); fy.append(fyi); x0.append(xi); fx.append(fxi)

    # ---- L0 bucket runs per chunk (bucket g = y0, rows (g, g+1)) -----------
    runs0 = [[] for _ in range(NCHUNK)]
    bk = y0[0]
    start = 0
    while start < N:
        g = bk[start]
        end = start
        while end < N and bk[end] == g:
            end += 1
        p = start
        while p < end:
            ci = p // CH
            q = min(end, (ci + 1) * CH)
            runs0[ci].append((int(g), p - ci * CH, q - p))
            p = q
        start = end

    g_lo, g_wid = [], []
    for s in range(NSUP):
        gs = [g for c in range(s * NCH_S, (s + 1) * NCH_S)
              for (g, off, ln) in runs0[c]]
        lo, hi = min(gs), max(gs)
        g_lo.append(int(lo)); g_wid.append(int(hi - lo + 1))
    w0max = max(g_wid)

    # ---- L1 / L2 window segments per chunk ---------------------------------
    seg1 = _windows(y0[1])
    seg2 = _windows(y0[2])

    # per-super block packing: local block index per segment
    def pack_blocks(segs):
        dev_segs = []   # per chunk: [(off, ln, local_block_idx)]
        blocks = []     # per super: [wbase,...]
        kcount, koff = [], []
        tot = 0
        for s in range(NSUP):
            blks = []
            for c in range(s * NCH_S, (s + 1) * NCH_S):
                lst = []
                for (off, ln, wb) in segs[c]:
                    lst.append((off, ln, len(blks)))
                    blks.append(wb)
                dev_segs.append(lst)
            blocks.append(blks)
            kcount.append(len(blks))
            koff.append(tot)
            tot += len(blks)
        return dev_segs, blocks, kcount, koff, tot

    dseg1, blocks1, k1, off1, tot1 = pack_blocks(seg1)
    dseg2, blocks2, k2, off2, tot2 = pack_blocks(seg2)
    k1max, k2max = max(k1), max(k2)
    fallback = any(len(s) != 1 or s[0][1] != CH for s in dseg1 + dseg2)

    # ---- S^T matrices [128, N] bf16 ----------------------------------------
    j = np.arange(N)
    wtl = [(1 - fy[l]) * (1 - fx[l]) for l in range(3)]
    wtr = [(1 - fy[l]) * fx[l] for l in range(3)]
    wbl = [fy[l] * (1 - fx[l]) for l in range(3)]
    wbr = [fy[l] * fx[l] for l in range(3)]

    def wbase_arr(segs):
        wb = np.zeros(N, np.int64)
        for c in range(NCHUNK):
            for (off, ln, w) in segs[c]:
                wb[c * CH + off: c * CH + off + ln] = w
        return wb

    wb1 = wbase_arr(seg1)
    wb2 = wbase_arr(seg2)

    s_t = []
    for l, ktop, kbot in (
        (0, x0[0], 64 + x0[0]),
        (1, (y0[1] - wb1) * 32 + x0[1], (y0[1] - wb1 + 1) * 32 + x0[1]),
        (2, (y0[2] - wb2) * 16 + x0[2], (y0[2] - wb2 + 1) * 16 + x0[2]),
    ):
        Sm = np.zeros((N, 128), np.float32)
        Sm[j, ktop] = wtl[l]
        Sm[j, ktop + 1] = wtr[l]
        Sm[j, kbot] = wbl[l]
        Sm[j, kbot + 1] = wbr[l]
        s_t.append(np.ascontiguousarray(Sm.T).astype(BF16))
    # enc values folded into s2t rows 64:106
    s2t = s_t[2].astype(np.float32)
    s2t[64:106] = _posenc_t(coords_s)
    s_t[2] = s2t.astype(BF16)

    # ---- pyramid + folded A grids ------------------------------------------
    R1 = _resize_matrix(32, 64)
    R2 = _resize_matrix(16, 64)
    g1 = np.einsum("ph,qw,bhwc->bpqc", R1, R1, fg, optimize=True)
    g2 = np.einsum("ph,qw,bhwc->bpqc", R2, R2, fg, optimize=True)

    def fold(grid, wslice):  # [B, h, w, C] @ [C, 256] -> [B, h, w, 256]
        hh, ww = grid.shape[1], grid.shape[2]
        return (grid.reshape(B, -1, C) @ wslice).reshape(B, hh, ww, 256)

    A00 = fold(fg, w0[42:298]);  A30 = fold(fg, w3[298:554])
    A01 = fold(g1, w0[298:554]); A31 = fold(g1, w3[554:810])
    A02 = fold(g2, w0[554:810]); A32 = fold(g2, w3[810:1066])
    enc0 = w0[0:42]              # [42, 256]
    enc3 = w3[256:298]

    # ---- per-core stationary tensors ---------------------------------------
    def rp0_pack(Ab):  # [64, 64, 256] -> [128, 63*256] (bucket g: rows g,g+1)
        st = np.stack([Ab[:-1], Ab[1:]], axis=1)        # [63, 2, 64, 256]
        return np.ascontiguousarray(
            st.transpose(1, 2, 0, 3).reshape(128, 63 * 256)).astype(BF16)

    def rp1_pack(Ab, blks):  # [32, 32, 256] -> [128, k*256] 4-row windows
        Ap = np.zeros((32 + 3, 32, 256), np.float32)
        Ap[:32] = Ab
        cols = [Ap[w:w + 4].reshape(128, 256) for w in blks]
        return np.ascontiguousarray(np.concatenate(cols, axis=1)).astype(BF16)

    def rp2_pack(Ab, blks, wenc):  # [16,16,256]: 4-row window + enc rows
        Ap = np.zeros((16 + 3, 16, 256), np.float32)
        Ap[:16] = Ab
        cols = []
        for w in blks:
            blk = np.zeros((128, 256), np.float32)
            blk[0:64] = Ap[w:w + 4].reshape(64, 256)
            blk[64:106] = wenc
            cols.append(blk)
        return np.ascontiguousarray(np.concatenate(cols, axis=1)).astype(BF16)

    flat1 = [w for s in range(NSUP) for w in blocks1[s]]
    flat2 = [w for s in range(NSUP) for w in blocks2[s]]
    per_core = []
    for b in range(B):
        per_core.append({
            "rp0a": rp0_pack(A00[b]), "rp0b": rp0_pack(A30[b]),
            "rp1a": rp1_pack(A01[b], flat1), "rp1b": rp1_pack(A31[b], flat1),
            "rp2a": rp2_pack(A02[b], flat2, enc0),
            "rp2b": rp2_pack(A32[b], flat2, enc3),
        })

    # ---- MLP weights / biases ----------------------------------------------
    def pack(wd):  # [Ktot, M] -> [128, (Ktot/128) * M], k-tile major
        K, M = wd.shape
        assert K % 128 == 0
        return np.ascontiguousarray(
            wd.reshape(K // 128, 128, M).transpose(1, 0, 2).reshape(128, -1)
        )

    woutd = np.zeros((256, 3), np.float32)
    woutd[:] = w_out

    shared = {
        "s0t": s_t[0], "s1t": s_t[1], "s2t": s_t[2],
        "w1": pack(w1).astype(BF16), "w2": pack(w2).astype(BF16),
        "w3h": pack(w3[0:256]).astype(BF16),
        "wout": pack(woutd).astype(BF16),
        "b0": np.asarray(b0, np.float32).reshape(2, 128).T.copy(),
        "b1": np.asarray(b1, np.float32).reshape(2, 128).T.copy(),
        "b2": np.asarray(b2, np.float32).reshape(2, 128).T.copy(),
        "b3": np.asarray(b3, np.float32).reshape(2, 128).T.copy(),
        "bout": np.asarray(b_out, np.float32).reshape(3, 1).copy(),
        "wz": np.zeros((128, 128), BF16),
    }

    meta = {
        "runs0": runs0, "g_lo": g_lo, "g_wid": g_wid, "w0max": w0max,
        "dseg1": dseg1, "k1": k1, "off1": off1, "tot1": tot1, "k1max": k1max,
        "dseg2": dseg2, "k2": k2, "off2": off2, "tot2": tot2, "k2max": k2max,
        "fallback": fallback,
    }
    return shared, per_core, perm, meta


def _build_nc(meta):
    """Build the Bacc program (shared by all cores; per-core data differs)."""
    from contextlib import ExitStack

    import concourse.bacc as bacc
    import concourse.mybir as mybir
    import concourse.tile as tile

    bf16 = mybir.dt.bfloat16
    f32 = mybir.dt.float32
    GELU = mybir.ActivationFunctionType.Gelu_apprx_tanh
    TANH = mybir.ActivationFunctionType.Tanh

    runs0 = meta["runs0"]
    g_lo, g_wid, w0max = meta["g_lo"], meta["g_wid"], meta["w0max"]
    dseg1, k1, off1, tot1, k1max = (meta["dseg1"], meta["k1"], meta["off1"],
                                    meta["tot1"], meta["k1max"])
    dseg2, k2, off2, tot2, k2max = (meta["dseg2"], meta["k2"], meta["off2"],
                                    meta["tot2"], meta["k2max"])

    nc = bacc.Bacc("TRN2", debug=False, target_bir_lowering=False)

    specs = [
        ("s0t", (128, N), bf16), ("s1t", (128, N), bf16),
        ("s2t", (128, N), bf16),
        ("rp0a", (128, 63 * 256), bf16), ("rp0b", (128, 63 * 256), bf16),
        ("rp1a", (128, tot1 * 256), bf16), ("rp1b", (128, tot1 * 256), bf16),
        ("rp2a", (128, tot2 * 256), bf16), ("rp2b", (128, tot2 * 256), bf16),
        ("w1", (128, 512), bf16), ("w2", (128, 512), bf16),
        ("w3h", (128, 512), bf16), ("wout", (128, 6), bf16),
        ("b0", (128, 2), f32), ("b1", (128, 2), f32),
        ("b2", (128, 2), f32), ("b3", (128, 2), f32),
        ("bout", (3, 1), f32), ("wz", (128, 128), bf16),
    ]
    dram = {}
    for name, shape, dt in specs:
        dram[name] = nc.dram_tensor(name, list(shape), dt, kind="ExternalInput")
    out_dram = nc.dram_tensor("out_t", [3, N], f32, kind="ExternalOutput")

    with tile.TileContext(nc) as tc, ExitStack() as ctx:
        const = ctx.enter_context(tc.tile_pool(name="const", bufs=1))
        spool = ctx.enter_context(tc.tile_pool(name="stream", bufs=2))
        rpool = ctx.enter_context(tc.tile_pool(name="rp", bufs=2))
        hpool = ctx.enter_context(tc.tile_pool(name="h", bufs=2))
        opool = ctx.enter_context(tc.tile_pool(name="osb", bufs=2))
        ps = ctx.enter_context(tc.tile_pool(name="ps", bufs=2, space="PSUM"))

        # ---- static tensors -------------------------------------------------
        st = {}
        for name in ("w1", "w2", "w3h", "wout", "b0", "b1", "b2", "b3",
                     "bout", "wz"):
            shape, dt = next((s, d) for n, s, d in specs if n == name)
            t = const.tile(list(shape), dt, tag=name)
            nc.sync.dma_start(t[:, :], dram[name][:, :])
            st[name] = t
        bmlp = [st["b0"], st["b1"], st["b2"], st["b3"]]

        for s in range(NSUP):
            lo = s * SUP
            sl = slice(lo, lo + SUP)

            s_tiles = []
            for nm in ("s0t", "s1t", "s2t"):
                t = spool.tile([128, SUP], bf16, tag=nm)
                nc.sync.dma_start(t[:, :], dram[nm][:, sl])
                s_tiles.append(t)
            s0, s1, s2 = s_tiles

            def rp_slice(nm, width, doff, tag):
                t = rpool.tile([128, width * 256], bf16, tag=tag)
                nc.sync.dma_start(t[:, :0 + (doff[1] - doff[0]) * 256],
                                  dram[nm][:, doff[0] * 256: doff[1] * 256])
                return t

            r0a = rp_slice("rp0a", w0max, (g_lo[s], g_lo[s] + g_wid[s]), "r0a")
            r0b = rp_slice("rp0b", w0max, (g_lo[s], g_lo[s] + g_wid[s]), "r0b")
            r1a = rp_slice("rp1a", k1max, (off1[s], off1[s] + k1[s]), "r1a")
            r1b = rp_slice("rp1b", k1max, (off1[s], off1[s] + k1[s]), "r1b")
            r2a = rp_slice("rp2a", k2max, (off2[s], off2[s] + k2[s]), "r2a")
            r2b = rp_slice("rp2b", k2max, (off2[s], off2[s] + k2[s]), "r2b")

            def sample_layer(P, m, r2, r1, r0, extra=None):
                """Accumulate one m-half of the fused sampling+enc (+extra)
                pre-activation into PSUM tile P [128, 2048]."""
                for ch in range(NCH_S):
                    c0 = ch * CH
                    csl = slice(c0, c0 + CH)
                    cidx = s * NCH_S + ch
                    # L2+enc: start anchor (full-width normal case)
                    sg2 = dseg2[cidx]
                    if len(sg2) == 1 and sg2[0][1] == CH:
                        off, ln, bi = sg2[0]
                        nc.tensor.matmul(
                            P[:, csl], r2[:, bi * 256 + m * 128:
                                          bi * 256 + m * 128 + 128],
                            s2[:, csl], start=True, stop=False)
                    else:
                        nc.tensor.matmul(P[:, csl], st["wz"][:, :],
                                         s0[:, csl], start=True, stop=False)
                        for (off, ln, bi) in sg2:
                            nc.tensor.matmul(
                                P[:, c0 + off: c0 + off + ln],
                                r2[:, bi * 256 + m * 128:
                                   bi * 256 + m * 128 + 128],
                                s2[:, c0 + off: c0 + off + ln],
                                start=False, stop=False)
                    if extra is not None:
                        extra(ch)
                    # L0 runs
                    for (g, off, ln) in runs0[cidx]:
                        nc.tensor.matmul(
                            P[:, c0 + off: c0 + off + ln],
                            r0[:, (g - g_lo[s]) * 256 + m * 128:
                               (g - g_lo[s]) * 256 + m * 128 + 128],
                            s0[:, c0 + off: c0 + off + ln],
                            start=False, stop=False)
                    # L1: stop anchor
                    sg1 = dseg1[cidx]
                    if len(sg1) == 1 and sg1[0][1] == CH:
                        off, ln, bi = sg1[0]
                        nc.tensor.matmul(
                            P[:, csl], r1[:, bi * 256 + m * 128:
                                          bi * 256 + m * 128 + 128],
                            s1[:, csl], start=False, stop=True)
                    else:
                        for (off, ln, bi) in sg1:
                            nc.tensor.matmul(
                                P[:, c0 + off: c0 + off + ln],
                                r1[:, bi * 256 + m * 128:
                                   bi * 256 + m * 128 + 128],
                                s1[:, c0 + off: c0 + off + ln],
                                start=False, stop=False)
                        nc.tensor.matmul(P[:, csl], st["wz"][:, :],
                                         s0[:, csl], start=False, stop=True)

            # ---- fused sampling + layer 0 ----------------------------------
            h0 = hpool.tile([128, 2 * SUP], bf16, tag="h0")
            for m in range(2):
                P = ps.tile([128, 2048], f32, tag="ps")
                sample_layer(P, m, r2a, r1a, r0a)
                nc.scalar.activation(h0[:, m * SUP:(m + 1) * SUP], P[:, :],
                                     GELU, bias=bmlp[0][:, m:m + 1])

            # ---- layers 1, 2 -----------------------------------------------
            def dense(wt, rhs, bias, outtag):
                h = hpool.tile([128, 2 * SUP], bf16, tag=outtag)
                for m in range(2):
                    P = ps.tile([128, 2048], f32, tag="ps")
                    for ch in range(NCH_S):
                        csl = slice(ch * CH, (ch + 1) * CH)
                        for kt in range(2):
                            nc.tensor.matmul(
                                P[:, csl],
                                wt[:, kt * 256 + m * 128:
                                   kt * 256 + m * 128 + 128],
                                rhs[:, kt * SUP + ch * CH:
                                    kt * SUP + (ch + 1) * CH],
                                start=(kt == 0), stop=(kt == 1))
                    nc.scalar.activation(h[:, m * SUP:(m + 1) * SUP], P[:, :],
                                         GELU, bias=bias[:, m:m + 1])
                return h

            h1 = dense(st["w1"], h0, bmlp[1], "h1")
            h2 = dense(st["w2"], h1, bmlp[2], "h2")

            # ---- layer 3 (skip concat folded) ------------------------------
            h3 = hpool.tile([128, 2 * SUP], bf16, tag="h3")
            for m in range(2):
                P = ps.tile([128, 2048], f32, tag="ps")

                def extra(ch, P=P, m=m):
                    csl = slice(ch * CH, (ch + 1) * CH)
                    for kt in range(2):
                        nc.tensor.matmul(
                            P[:, csl],
                            st["w3h"][:, kt * 256 + m * 128:
                                      kt * 256 + m * 128 + 128],
                            h2[:, kt * SUP + ch * CH: kt * SUP + (ch + 1) * CH],
                            start=False, stop=False)

                sample_layer(P, m, r2b, r1b, r0b, extra=extra)
                nc.scalar.activation(h3[:, m * SUP:(m + 1) * SUP], P[:, :],
                                     GELU, bias=bmlp[3][:, m:m + 1])

            # ---- output layer ----------------------------------------------
            osb = opool.tile([3, SUP], f32, tag="osb")
            PO = ps.tile([128, 2048], f32, tag="ps")
            for ch in range(NCH_S):
                csl = slice(ch * CH, (ch + 1) * CH)
                for kt in range(2):
                    nc.tensor.matmul(
                        PO[:3, csl],
                        st["wout"][:, kt * 3:(kt + 1) * 3],
                        h3[:, kt * SUP + ch * CH: kt * SUP + (ch + 1) * CH],
                        start=(kt == 0), stop=(kt == 1))
            nc.scalar.activation(osb[:, :], PO[:3, :], TANH,
                                 bias=st["bout"][:, 0:1])
            nc.sync.dma_start(out_dram[:, sl], osb[:, :])

    nc.compile()
    return nc


def kernel(feature_grid, coords, w0, b0, w1, b1, w2, b2, w3, b3, w_out, b_out,
           _run_opts=None):
    from concourse.bass_utils import run_bass_kernel_spmd

    shared, per_core, perm, meta = _host_prep(
        feature_grid, coords, w0, b0, w1, b1, w2, b2, w3, b3, w_out, b_out)

    nc = _build_nc(meta)

    in_maps = []
    for b in range(B):
        m = dict(shared)
        m.update(per_core[b])
        in_maps.append(m)

    res = run_bass_kernel_spmd(
        nc, in_maps, core_ids=list(range(B)), **(_run_opts or {})
    )

    out = np.empty((B, N, 3), np.float32)
    for b in range(B):
        out[b, perm, :] = res.results[b]["out_t"].T
    if _run_opts is not None:
        kernel._last_result = res  # for test harness introspection
    return out


# revision 7
# speedup vs baseline: 1.7563x; 1.2656x over previous
"""Trainium2 Bass kernel for nn_CoordinateDecoder.

Computation (see reference): posenc(coords) ++ bilinear-pyramid-sampled
features -> 5-layer MLP (gelu-tanh approx, skip concat at depth 2, tanh out).

Strategy:
  - Data-parallel over B: core b handles batch image b (coords/weights shared).
  - KEY TRICK: bilinear sampling is linear, so the sampled features' MLP
    contribution folds into the grids on the host:
        feats @ W0_lvl = S_lvl^T (G_lvl @ W0_lvl) = S_lvl^T A_lvl .
    The on-device sampling matmuls therefore emit layer-0 (and layer-3 skip)
    PRE-ACTIVATIONS directly, accumulated in PSUM across pyramid levels.
    This removes the layer-0 GEMM (7 k-tiles) and 6 of 9 k-tiles of layer 3.
  - Samples are host-sorted by continuous y.  Per 512-sample chunk:
      L0 (64x64 grid): row-pair bucket runs (matmul per run, k=128 cells).
      L1 (32x32 grid): one full-width matmul, k = 4-grid-row window (4x32).
      L2 (16x16 grid): one full-width matmul, k = 4-row window (4x16=64 cells)
        + the 42 posenc dims folded into spare k-rows 64:106 (enc weight rows
        live in the L2 stationary block; enc values live in s2t rows 64:106).
    The full-width L2 matmul is the PSUM bank's start=True anchor and the
    full-width L1 matmul its stop=True anchor; partial L0-run matmuls are
    ordered between them by byte-range WAW deps (start=True pending-zeroes the
    whole bank; start=False overwrites on first touch, accumulates after).
  - MLP in bf16 (fp32 PSUM), gelu fused on ScalarE as one [128,2048] call per
    (layer, m-half) spanning 4 PSUM banks to amortize ACT overhead.
  - Host does O(N)/O(grid) prep: pyramid resize, A = G @ W folds (~11 GFLOP
    BLAS), posenc, bilinear indices, argsort, packing.
"""

import numpy as np
import ml_dtypes

BF16 = ml_dtypes.bfloat16

B, H, W, C = 8, 64, 64, 256
N = 16384
NUM_FREQS = 10
MLP_WIDTH = 256

NSUP = 8            # column supers
SUP = N // NSUP     # 2048
CH = 512            # psum-bank chunk
NCH_S = SUP // CH   # 4 chunks per super
NCHUNK = N // CH    # 32

LEVEL_SIZES = [64, 32, 16]


def _resize_matrix(out_size: int, in_size: int) -> np.ndarray:
    """Row-resize operator of jax.image.resize(..., 'bilinear') (antialias).
    Returns M [out, in] with resized = M @ x."""
    scale = out_size / in_size
    inv_scale = 1.0 / scale
    kernel_scale = max(inv_scale, 1.0)
    sample_f = (np.arange(out_size, dtype=np.float64) + 0.5) * inv_scale - 0.5
    x = np.abs(sample_f[None, :] - np.arange(in_size, dtype=np.float64)[:, None])
    x = x / kernel_scale
    w = np.where(x < 1.0, 1.0 - x, 0.0)
    total = w.sum(axis=0, keepdims=True)
    w = np.where(
        np.abs(total) > 1000.0 * np.finfo(np.float32).eps,
        w / np.where(total != 0.0, total, 1.0),
        0.0,
    )
    w = np.where(
        ((sample_f >= -0.5) & (sample_f <= in_size - 0.5))[None, :], w, 0.0
    )
    return w.T.astype(np.float32)  # [out, in]


def _posenc_t(coords: np.ndarray) -> np.ndarray:
    """Transposed positional encoding [42, n] fp32, matching reference order."""
    freqs = (2.0 ** np.arange(NUM_FREQS, dtype=np.float32)) * np.float32(np.pi)
    parts = [coords.T.astype(np.float32)]
    for f in freqs:
        parts.append(np.sin(coords.T * f).astype(np.float32))
        parts.append(np.cos(coords.T * f).astype(np.float32))
    return np.concatenate(parts, axis=0)  # [42, n]


def _bilinear(c01: np.ndarray, size: int):
    """c01 [n] in [0,1] -> (i0, frac) fp32 like the reference's fp32 math."""
    cr = (c01 * np.float32(size - 1)).astype(np.float32)
    i0 = np.floor(cr).astype(np.int64)
    i0 = np.clip(i0, 0, size - 2)
    f = cr - i0.astype(np.float32)
    return i0, f.astype(np.float32)


def _windows(ys: np.ndarray):
    """Per 512-chunk segments [(off, ln, wbase)] s.t. all y0 in a segment fit
    rows [wbase, wbase+4) (y0 and y0+1 both inside).  ys nondecreasing."""
    segs = []
    for c in range(NCHUNK):
        yc = ys[c * CH:(c + 1) * CH]
        lst = []
        p = 0
        while p < CH:
            wb = int(yc[p])
            q = p
            while q < CH and yc[q] <= wb + 2:
                q += 1
            lst.append((p, q - p, wb))
            p = q
        segs.append(lst)
    return segs


def _host_prep(feature_grid, coords, w0, b0, w1, b1, w2, b2, w3, b3, w_out, b_out):
    fg = np.asarray(feature_grid, dtype=np.float32)
    coords = np.asarray(coords, dtype=np.float32)
    w0 = np.asarray(w0, np.float32); w1 = np.asarray(w1, np.float32)
    w2 = np.asarray(w2, np.float32); w3 = np.asarray(w3, np.float32)
    w_out = np.asarray(w_out, np.float32)

    # ---- sort samples by continuous y --------------------------------------
    c01 = (coords + np.float32(1.0)) / np.float32(2.0)  # [N,2] (y, x)
    perm = np.argsort(c01[:, 0], kind="stable")
    c01s = c01[perm]
    coords_s = coords[perm]

    y0, fy, x0, fx = [], [], [], []
    for S in LEVEL_SIZES:
        yi, fyi = _bilinear(c01s[:, 0], S)
        xi, fxi = _bilinear(c01s[:, 1], S)
        y0.append(yi); fy.append(fyi); x0.append(xi); fx.append(fxi)

    # ---- L0 bucket runs per chunk (bucket g = y0, rows (g, g+1)) -----------
    runs0 = [[] for _ in range(NCHUNK)]
    bk = y0[0]
    start = 0
    while start < N:
        g = bk[start]
        end = start
        while end < N and bk[end] == g:
            end += 1
        p = start
        while p < end:
            ci = p // CH
            q = min(end, (ci + 1) * CH)
            runs0[ci].append((int(g), p - ci * CH, q - p))
            p = q
        start = end

    g_lo, g_wid = [], []
    for s in range(NSUP):
        gs = [g for c in range(s * NCH_S, (s + 1) * NCH_S)
              for (g, off, ln) in runs0[c]]
        lo, hi = min(gs), max(gs)
        g_lo.append(int(lo)); g_wid.append(int(hi - lo + 1))
    w0max = max(g_wid)

    # ---- L1 / L2 window segments per chunk ---------------------------------
    seg1 = _windows(y0[1])
    seg2 = _windows(y0[2])

    # per-super block packing: local block index per segment
    def pack_blocks(segs):
        dev_segs = []   # per chunk: [(off, ln, local_block_idx)]
        blocks = []     # per super: [wbase,...]
        kcount, koff = [], []
        tot = 0
        for s in range(NSUP):
            blks = []
            for c in range(s * NCH_S, (s + 1) * NCH_S):
                lst = []
                for (off, ln, wb) in segs[c]:
                    lst.append((off, ln, len(blks)))
                    blks.append(wb)
                dev_segs.append(lst)
            blocks.append(blks)
            kcount.append(len(blks))
            koff.append(tot)
            tot += len(blks)
        return dev_segs, blocks, kcount, koff, tot

    dseg1, blocks1, k1, off1, tot1 = pack_blocks(seg1)
    dseg2, blocks2, k2, off2, tot2 = pack_blocks(seg2)
    k1max, k2max = max(k1), max(k2)
    fallback = any(len(s) != 1 or s[0][1] != CH for s in dseg1 + dseg2)

    # ---- S^T matrices [128, N] bf16 ----------------------------------------
    j = np.arange(N)
    wtl = [(1 - fy[l]) * (1 - fx[l]) for l in range(3)]
    wtr = [(1 - fy[l]) * fx[l] for l in range(3)]
    wbl = [fy[l] * (1 - fx[l]) for l in range(3)]
    wbr = [fy[l] * fx[l] for l in range(3)]

    def wbase_arr(segs):
        wb = np.zeros(N, np.int64)
        for c in range(NCHUNK):
            for (off, ln, w) in segs[c]:
                wb[c * CH + off: c * CH + off + ln] = w
        return wb

    wb1 = wbase_arr(seg1)
    wb2 = wbase_arr(seg2)

    s_t = []
    for l, ktop, kbot in (
        (0, x0[0], 64 + x0[0]),
        (1, (y0[1] - wb1) * 32 + x0[1], (y0[1] - wb1 + 1) * 32 + x0[1]),
        (2, (y0[2] - wb2) * 16 + x0[2], (y0[2] - wb2 + 1) * 16 + x0[2]),
    ):
        Sm = np.zeros((N, 128), np.float32)
        Sm[j, ktop] = wtl[l]
        Sm[j, ktop + 1] = wtr[l]
        Sm[j, kbot] = wbl[l]
        Sm[j, kbot + 1] = wbr[l]
        s_t.append(np.ascontiguousarray(Sm.T).astype(BF16))
    # enc values folded into s2t rows 64:106
    s2t = s_t[2].astype(np.float32)
    s2t[64:106] = _posenc_t(coords_s)
    s_t[2] = s2t.astype(BF16)

    # ---- pyramid + folded A grids ------------------------------------------
    R1 = _resize_matrix(32, 64)
    R2 = _resize_matrix(16, 64)
    g1 = np.einsum("ph,qw,bhwc->bpqc", R1, R1, fg, optimize=True)
    g2 = np.einsum("ph,qw,bhwc->bpqc", R2, R2, fg, optimize=True)

    def fold(grid, wslice):  # [B, h, w, C] @ [C, 256] -> [B, h, w, 256]
        hh, ww = grid.shape[1], grid.shape[2]
        return (grid.reshape(B, -1, C) @ wslice).reshape(B, hh, ww, 256)

    A00 = fold(fg, w0[42:298]);  A30 = fold(fg, w3[298:554])
    A01 = fold(g1, w0[298:554]); A31 = fold(g1, w3[554:810])
    A02 = fold(g2, w0[554:810]); A32 = fold(g2, w3[810:1066])
    enc0 = w0[0:42]              # [42, 256]
    enc3 = w3[256:298]

    # ---- per-core stationary tensors ---------------------------------------
    def rp0_pack(Ab):  # [64, 64, 256] -> [128, 63*256] (bucket g: rows g,g+1)
        st = np.stack([Ab[:-1], Ab[1:]], axis=1)        # [63, 2, 64, 256]
        return np.ascontiguousarray(
            st.transpose(1, 2, 0, 3).reshape(128, 63 * 256)).astype(BF16)

    def rp1_pack(Ab, blks):  # [32, 32, 256] -> [128, k*256] 4-row windows
        Ap = np.zeros((32 + 3, 32, 256), np.float32)
        Ap[:32] = Ab
        cols = [Ap[w:w + 4].reshape(128, 256) for w in blks]
        return np.ascontiguousarray(np.concatenate(cols, axis=1)).astype(BF16)

    def rp2_pack(Ab, blks, wenc):  # [16,16,256]: 4-row window + enc rows
        Ap = np.zeros((16 + 3, 16, 256), np.float32)
        Ap[:16] = Ab
        cols = []
        for w in blks:
            blk = np.zeros((128, 256), np.float32)
            blk[0:64] = Ap[w:w + 4].reshape(64, 256)
            blk[64:106] = wenc
            cols.append(blk)
        return np.ascontiguousarray(np.concatenate(cols, axis=1)).astype(BF16)

    flat1 = [w for s in range(NSUP) for w in blocks1[s]]
    flat2 = [w for s in range(NSUP) for w in blocks2[s]]
    per_core = []
    for b in range(B):
        per_core.append({
            "rp0a": rp0_pack(A00[b]), "rp0b": rp0_pack(A30[b]),
            "rp1a": rp1_pack(A01[b], flat1), "rp1b": rp1_pack(A31[b], flat1),
            "rp2a": rp2_pack(A02[b], flat2, enc0),
            "rp2b": rp2_pack(A32[b], flat2, enc3),
        })

    # ---- MLP weights / biases ----------------------------------------------
    def pack(wd):  # [Ktot, M] -> [128, (Ktot/128) * M], k-tile major
        K, M = wd.shape
        assert K % 128 == 0
        return np.ascontiguousarray(
            wd.reshape(K // 128, 128, M).transpose(1, 0, 2).reshape(128, -1)
        )

    woutd = np.zeros((256, 3), np.float32)
    woutd[:] = w_out

    shared = {
        "s0t": s_t[0], "s1t": s_t[1], "s2t": s_t[2],
        "w1": pack(w1).astype(BF16), "w2": pack(w2).astype(BF16),
        "w3h": pack(w3[0:256]).astype(BF16),
        "wout": pack(woutd).astype(BF16),
        "b0": np.asarray(b0, np.float32).reshape(2, 128).T.copy(),
        "b1": np.asarray(b1, np.float32).reshape(2, 128).T.copy(),
        "b2": np.asarray(b2, np.float32).reshape(2, 128).T.copy(),
        "b3": np.asarray(b3, np.float32).reshape(2, 128).T.copy(),
        "bout": np.asarray(b_out, np.float32).reshape(3, 1).copy(),
        "wz": np.zeros((128, 128), BF16),
    }

    meta = {
        "runs0": runs0, "g_lo": g_lo, "g_wid": g_wid, "w0max": w0max,
        "dseg1": dseg1, "k1": k1, "off1": off1, "tot1": tot1, "k1max": k1max,
        "dseg2": dseg2, "k2": k2, "off2": off2, "tot2": tot2, "k2max": k2max,
        "fallback": fallback,
    }
    return shared, per_core, perm, meta


def _build_nc(meta):
    """Build the Bacc program (shared by all cores; per-core data differs)."""
    from contextlib import ExitStack

    import concourse.bacc as bacc
    import concourse.mybir as mybir
    import concourse.tile as tile

    bf16 = mybir.dt.bfloat16
    f32 = mybir.dt.float32
    GELU = mybir.ActivationFunctionType.Gelu_apprx_tanh
    TANH = mybir.ActivationFunctionType.Tanh

    runs0 = meta["runs0"]
    g_lo, g_wid, w0max = meta["g_lo"], meta["g_wid"], meta["w0max"]
    dseg1, k1, off1, tot1, k1max = (meta["dseg1"], meta["k1"], meta["off1"],
                                    meta["tot1"], meta["k1max"])
    dseg2, k2, off2, tot2, k2max = (meta["dseg2"], meta["k2"], meta["off2"],
                                    meta["tot2"], meta["k2max"])

    nc = bacc.Bacc("TRN2", debug=False, target_bir_lowering=False)

    specs = [
        ("s0t", (128, N), bf16), ("s1t", (128, N), bf16),
        ("s2t", (128, N), bf16),
        ("rp0a", (128, 63 * 256), bf16), ("rp0b", (128, 63 * 256), bf16),
        ("rp1a", (128, tot1 * 256), bf16), ("rp1b", (128, tot1 * 256), bf16),
        ("rp2a", (128, tot2 * 256), bf16), ("rp2b", (128, tot2 * 256), bf16),
        ("w1", (128, 512), bf16), ("w2", (128, 512), bf16),
        ("w3h", (128, 512), bf16), ("wout", (128, 6), bf16),
        ("b0", (128, 2), f32), ("b1", (128, 2), f32),
        ("b2", (128, 2), f32), ("b3", (128, 2), f32),
        ("bout", (3, 1), f32), ("wz", (128, 128), bf16),
    ]
    dram = {}
    for name, shape, dt in specs:
        dram[name] = nc.dram_tensor(name, list(shape), dt, kind="ExternalInput")
    out_dram = nc.dram_tensor("out_t", [3, N], f32, kind="ExternalOutput")

    with tile.TileContext(nc) as tc, ExitStack() as ctx:
        const = ctx.enter_context(tc.tile_pool(name="const", bufs=1))
        spool = ctx.enter_context(tc.tile_pool(name="stream", bufs=2))
        rpool = ctx.enter_context(tc.tile_pool(name="rp", bufs=2))
        hpool = ctx.enter_context(tc.tile_pool(name="h", bufs=2))
        opool = ctx.enter_context(tc.tile_pool(name="osb", bufs=2))
        ps = ctx.enter_context(tc.tile_pool(name="ps", bufs=4, space="PSUM"))

        # ---- static tensors -------------------------------------------------
        st = {}
        for name in ("w1", "w2", "w3h", "wout", "b0", "b1", "b2", "b3",
                     "bout", "wz"):
            shape, dt = next((s, d) for n, s, d in specs if n == name)
            t = const.tile(list(shape), dt, tag=name)
            nc.sync.dma_start(t[:, :], dram[name][:, :])
            st[name] = t
        bmlp = [st["b0"], st["b1"], st["b2"], st["b3"]]

        for s in range(NSUP):
            lo = s * SUP
            sl = slice(lo, lo + SUP)

            # stream tiles: split DMAs per 512-chunk across queues; the
            # critical-path s2 (sampling start anchor) pieces go out first.
            def stream_tile(nm):
                t = spool.tile([128, SUP], bf16, tag=nm)
                for c in range(NCH_S):
                    nc.sync.dma_start(
                        t[:, c * CH:(c + 1) * CH],
                        dram[nm][:, lo + c * CH: lo + (c + 1) * CH])
                return t

            def rp_slice(nm, width, doff, tag):
                t = rpool.tile([128, width * 256], bf16, tag=tag)
                nblk = doff[1] - doff[0]
                step = 2  # blocks per DMA piece (spread across queues)
                for b0 in range(0, nblk, step):
                    b1 = min(b0 + step, nblk)
                    nc.sync.dma_start(
                        t[:, b0 * 256: b1 * 256],
                        dram[nm][:, (doff[0] + b0) * 256:
                                 (doff[0] + b1) * 256])
                return t

            s2 = stream_tile("s2t")
            r2a = rp_slice("rp2a", k2max, (off2[s], off2[s] + k2[s]), "r2a")
            s0 = stream_tile("s0t")
            r0a = rp_slice("rp0a", w0max, (g_lo[s], g_lo[s] + g_wid[s]), "r0a")
            s1 = stream_tile("s1t")
            r1a = rp_slice("rp1a", k1max, (off1[s], off1[s] + k1[s]), "r1a")
            r2b = rp_slice("rp2b", k2max, (off2[s], off2[s] + k2[s]), "r2b")
            r0b = rp_slice("rp0b", w0max, (g_lo[s], g_lo[s] + g_wid[s]), "r0b")
            r1b = rp_slice("rp1b", k1max, (off1[s], off1[s] + k1[s]), "r1b")

            def sample_layer(P, m, hf, r2, r1, r0, extra=None):
                """Accumulate chunks (2*hf, 2*hf+1) of one m-half of the fused
                sampling+enc (+extra) pre-activation into PSUM tile P
                [128, 1024]."""
                for ch in range(2 * hf, 2 * hf + 2):
                    c0 = (ch - 2 * hf) * CH     # offset within P
                    g0 = ch * CH                # offset within super streams
                    psl = slice(c0, c0 + CH)
                    gsl = slice(g0, g0 + CH)
                    cidx = s * NCH_S + ch
                    # L2+enc: start anchor (full-width normal case)
                    sg2 = dseg2[cidx]
                    if len(sg2) == 1 and sg2[0][1] == CH:
                        off, ln, bi = sg2[0]
                        nc.tensor.matmul(
                            P[:, psl], r2[:, bi * 256 + m * 128:
                                          bi * 256 + m * 128 + 128],
                            s2[:, gsl], start=True, stop=False)
                    else:
                        nc.tensor.matmul(P[:, psl], st["wz"][:, :],
                                         s0[:, gsl], start=True, stop=False)
                        for (off, ln, bi) in sg2:
                            nc.tensor.matmul(
                                P[:, c0 + off: c0 + off + ln],
                                r2[:, bi * 256 + m * 128:
                                   bi * 256 + m * 128 + 128],
                                s2[:, g0 + off: g0 + off + ln],
                                start=False, stop=False)
                    if extra is not None:
                        extra(ch)
                    # L0 runs
                    for (g, off, ln) in runs0[cidx]:
                        nc.tensor.matmul(
                            P[:, c0 + off: c0 + off + ln],
                            r0[:, (g - g_lo[s]) * 256 + m * 128:
                               (g - g_lo[s]) * 256 + m * 128 + 128],
                            s0[:, g0 + off: g0 + off + ln],
                            start=False, stop=False)
                    # L1: stop anchor
                    sg1 = dseg1[cidx]
                    if len(sg1) == 1 and sg1[0][1] == CH:
                        off, ln, bi = sg1[0]
                        nc.tensor.matmul(
                            P[:, psl], r1[:, bi * 256 + m * 128:
                                          bi * 256 + m * 128 + 128],
                            s1[:, gsl], start=False, stop=True)
                    else:
                        for (off, ln, bi) in sg1:
                            nc.tensor.matmul(
                                P[:, c0 + off: c0 + off + ln],
                                r1[:, bi * 256 + m * 128:
                                   bi * 256 + m * 128 + 128],
                                s1[:, g0 + off: g0 + off + ln],
                                start=False, stop=False)
                        nc.tensor.matmul(P[:, psl], st["wz"][:, :],
                                         s0[:, gsl], start=False, stop=True)

            HF = 1024

            # ---- fused sampling + layer 0 ----------------------------------
            h0 = hpool.tile([128, 2 * SUP], bf16, tag="h0")
            for m in range(2):
                for hf in range(2):
                    P = ps.tile([128, HF], f32, tag="ps")
                    sample_layer(P, m, hf, r2a, r1a, r0a)
                    nc.scalar.activation(
                        h0[:, m * SUP + hf * HF: m * SUP + (hf + 1) * HF],
                        P[:, :], GELU, bias=bmlp[0][:, m:m + 1])

            # ---- layers 1, 2 -----------------------------------------------
            def dense(wt, rhs, bias, outtag):
                h = hpool.tile([128, 2 * SUP], bf16, tag=outtag)
                for m in range(2):
                    for hf in range(2):
                        P = ps.tile([128, HF], f32, tag="ps")
                        for ch in range(2 * hf, 2 * hf + 2):
                            psl = slice((ch - 2 * hf) * CH,
                                        (ch - 2 * hf + 1) * CH)
                            for kt in range(2):
                                nc.tensor.matmul(
                                    P[:, psl],
                                    wt[:, kt * 256 + m * 128:
                                       kt * 256 + m * 128 + 128],
                                    rhs[:, kt * SUP + ch * CH:
                                        kt * SUP + (ch + 1) * CH],
                                    start=(kt == 0), stop=(kt == 1))
                        nc.scalar.activation(
                            h[:, m * SUP + hf * HF: m * SUP + (hf + 1) * HF],
                            P[:, :], GELU, bias=bias[:, m:m + 1])
                return h

            h1 = dense(st["w1"], h0, bmlp[1], "h1")
            h2 = dense(st["w2"], h1, bmlp[2], "h2")

            # ---- layer 3 (skip concat folded) ------------------------------
            h3 = hpool.tile([128, 2 * SUP], bf16, tag="h3")
            for m in range(2):
                for hf in range(2):
                    P = ps.tile([128, HF], f32, tag="ps")

                    def extra(ch, P=P, m=m, hf=hf):
                        psl = slice((ch - 2 * hf) * CH, (ch - 2 * hf + 1) * CH)
                        for kt in range(2):
                            nc.tensor.matmul(
                                P[:, psl],
                                st["w3h"][:, kt * 256 + m * 128:
                                          kt * 256 + m * 128 + 128],
                                h2[:, kt * SUP + ch * CH:
                                   kt * SUP + (ch + 1) * CH],
                                start=False, stop=False)

                    sample_layer(P, m, hf, r2b, r1b, r0b, extra=extra)
                    nc.scalar.activation(
                        h3[:, m * SUP + hf * HF: m * SUP + (hf + 1) * HF],
                        P[:, :], GELU, bias=bmlp[3][:, m:m + 1])

            # ---- output layer ----------------------------------------------
            osb = opool.tile([3, SUP], f32, tag="osb")
            for hf in range(2):
                PO = ps.tile([128, HF], f32, tag="ps")
                for ch in range(2 * hf, 2 * hf + 2):
                    psl = slice((ch - 2 * hf) * CH, (ch - 2 * hf + 1) * CH)
                    for kt in range(2):
                        nc.tensor.matmul(
                            PO[:3, psl],
                            st["wout"][:, kt * 3:(kt + 1) * 3],
                            h3[:, kt * SUP + ch * CH: kt * SUP + (ch + 1) * CH],
                            start=(kt == 0), stop=(kt == 1))
                nc.scalar.activation(osb[:, hf * HF:(hf + 1) * HF],
                                     PO[:3, :], TANH, bias=st["bout"][:, 0:1])
            nc.sync.dma_start(out_dram[:, sl], osb[:, :])

    nc.compile()
    return nc


def kernel(feature_grid, coords, w0, b0, w1, b1, w2, b2, w3, b3, w_out, b_out,
           _run_opts=None):
    from concourse.bass_utils import run_bass_kernel_spmd

    shared, per_core, perm, meta = _host_prep(
        feature_grid, coords, w0, b0, w1, b1, w2, b2, w3, b3, w_out, b_out)

    nc = _build_nc(meta)

    in_maps = []
    for b in range(B):
        m = dict(shared)
        m.update(per_core[b])
        in_maps.append(m)

    res = run_bass_kernel_spmd(
        nc, in_maps, core_ids=list(range(B)), **(_run_opts or {})
    )

    out = np.empty((B, N, 3), np.float32)
    for b in range(B):
        out[b, perm, :] = res.results[b]["out_t"].T
    if _run_opts is not None:
        kernel._last_result = res  # for test harness introspection
    return out


# revision 9
# speedup vs baseline: 1.8864x; 1.0741x over previous
"""Trainium2 Bass kernel for nn_CoordinateDecoder.

Computation (see reference): posenc(coords) ++ bilinear-pyramid-sampled
features -> 5-layer MLP (gelu-tanh approx, skip concat at depth 2, tanh out).

Strategy:
  - Data-parallel over B: core b handles batch image b (coords/weights shared).
  - KEY TRICK: bilinear sampling is linear, so the sampled features' MLP
    contribution folds into the grids on the host:
        feats @ W0_lvl = S_lvl^T (G_lvl @ W0_lvl) = S_lvl^T A_lvl .
    The on-device sampling matmuls therefore emit layer-0 (and layer-3 skip)
    PRE-ACTIVATIONS directly, accumulated in PSUM across pyramid levels.
    This removes the layer-0 GEMM (7 k-tiles) and 6 of 9 k-tiles of layer 3.
  - Samples are host-sorted by continuous y.  Per 512-sample chunk:
      L0 (64x64 grid): row-pair bucket runs (matmul per run, k=128 cells).
      L1 (32x32 grid): one full-width matmul, k = 4-grid-row window (4x32).
      L2 (16x16 grid): one full-width matmul, k = 4-row window (4x16=64 cells)
        + the 42 posenc dims folded into spare k-rows 64:106 (enc weight rows
        live in the L2 stationary block; enc values live in s2t rows 64:106).
    The full-width L2 matmul is the PSUM bank's start=True anchor and the
    full-width L1 matmul its stop=True anchor; partial L0-run matmuls are
    ordered between them by byte-range WAW deps (start=True pending-zeroes the
    whole bank; start=False overwrites on first touch, accumulates after).
  - MLP in bf16 (fp32 PSUM), gelu fused on ScalarE as one [128,2048] call per
    (layer, m-half) spanning 4 PSUM banks to amortize ACT overhead.
  - Host does O(N)/O(grid) prep: pyramid resize, A = G @ W folds (~11 GFLOP
    BLAS), posenc, bilinear indices, argsort, packing.
"""

import numpy as np
import ml_dtypes

BF16 = ml_dtypes.bfloat16

B, H, W, C = 8, 64, 64, 256
N = 16384
NUM_FREQS = 10
MLP_WIDTH = 256

NSUP = 8            # column supers
SUP = N // NSUP     # 2048
CH = 512            # psum-bank chunk
NCH_S = SUP // CH   # 4 chunks per super
NCHUNK = N // CH    # 32

LEVEL_SIZES = [64, 32, 16]


def _resize_matrix(out_size: int, in_size: int) -> np.ndarray:
    """Row-resize operator of jax.image.resize(..., 'bilinear') (antialias).
    Returns M [out, in] with resized = M @ x."""
    scale = out_size / in_size
    inv_scale = 1.0 / scale
    kernel_scale = max(inv_scale, 1.0)
    sample_f = (np.arange(out_size, dtype=np.float64) + 0.5) * inv_scale - 0.5
    x = np.abs(sample_f[None, :] - np.arange(in_size, dtype=np.float64)[:, None])
    x = x / kernel_scale
    w = np.where(x < 1.0, 1.0 - x, 0.0)
    total = w.sum(axis=0, keepdims=True)
    w = np.where(
        np.abs(total) > 1000.0 * np.finfo(np.float32).eps,
        w / np.where(total != 0.0, total, 1.0),
        0.0,
    )
    w = np.where(
        ((sample_f >= -0.5) & (sample_f <= in_size - 0.5))[None, :], w, 0.0
    )
    return w.T.astype(np.float32)  # [out, in]


def _posenc_t(coords: np.ndarray) -> np.ndarray:
    """Transposed positional encoding [42, n] fp32, matching reference order."""
    freqs = (2.0 ** np.arange(NUM_FREQS, dtype=np.float32)) * np.float32(np.pi)
    parts = [coords.T.astype(np.float32)]
    for f in freqs:
        parts.append(np.sin(coords.T * f).astype(np.float32))
        parts.append(np.cos(coords.T * f).astype(np.float32))
    return np.concatenate(parts, axis=0)  # [42, n]


def _bilinear(c01: np.ndarray, size: int):
    """c01 [n] in [0,1] -> (i0, frac) fp32 like the reference's fp32 math."""
    cr = (c01 * np.float32(size - 1)).astype(np.float32)
    i0 = np.floor(cr).astype(np.int64)
    i0 = np.clip(i0, 0, size - 2)
    f = cr - i0.astype(np.float32)
    return i0, f.astype(np.float32)


def _windows(ys: np.ndarray):
    """Per 512-chunk segments [(off, ln, wbase)] s.t. all y0 in a segment fit
    rows [wbase, wbase+4) (y0 and y0+1 both inside).  ys nondecreasing."""
    segs = []
    for c in range(NCHUNK):
        yc = ys[c * CH:(c + 1) * CH]
        lst = []
        p = 0
        while p < CH:
            wb = int(yc[p])
            q = p
            while q < CH and yc[q] <= wb + 2:
                q += 1
            lst.append((p, q - p, wb))
            p = q
        segs.append(lst)
    return segs


def _host_prep(feature_grid, coords, w0, b0, w1, b1, w2, b2, w3, b3, w_out, b_out):
    fg = np.asarray(feature_grid, dtype=np.float32)
    coords = np.asarray(coords, dtype=np.float32)
    w0 = np.asarray(w0, np.float32); w1 = np.asarray(w1, np.float32)
    w2 = np.asarray(w2, np.float32); w3 = np.asarray(w3, np.float32)
    w_out = np.asarray(w_out, np.float32)

    # ---- sort samples by continuous y --------------------------------------
    c01 = (coords + np.float32(1.0)) / np.float32(2.0)  # [N,2] (y, x)
    perm = np.argsort(c01[:, 0], kind="stable")
    c01s = c01[perm]
    coords_s = coords[perm]

    y0, fy, x0, fx = [], [], [], []
    for S in LEVEL_SIZES:
        yi, fyi = _bilinear(c01s[:, 0], S)
        xi, fxi = _bilinear(c01s[:, 1], S)
        y0.append(yi); fy.append(fyi); x0.append(xi); fx.append(fxi)

    # ---- L0 bucket runs per chunk (bucket g = y0, rows (g, g+1)) -----------
    runs0 = [[] for _ in range(NCHUNK)]
    bk = y0[0]
    start = 0
    while start < N:
        g = bk[start]
        end = start
        while end < N and bk[end] == g:
            end += 1
        p = start
        while p < end:
            ci = p // CH
            q = min(end, (ci + 1) * CH)
            runs0[ci].append((int(g), p - ci * CH, q - p))
            p = q
        start = end

    g_lo, g_wid = [], []
    for s in range(NSUP):
        gs = [g for c in range(s * NCH_S, (s + 1) * NCH_S)
              for (g, off, ln) in runs0[c]]
        lo, hi = min(gs), max(gs)
        g_lo.append(int(lo)); g_wid.append(int(hi - lo + 1))
    w0max = max(g_wid)

    # ---- L1 / L2 window segments per chunk ---------------------------------
    seg1 = _windows(y0[1])
    seg2 = _windows(y0[2])

    # per-super block packing: local block index per segment
    def pack_blocks(segs):
        dev_segs = []   # per chunk: [(off, ln, local_block_idx)]
        blocks = []     # per super: [wbase,...]
        kcount, koff = [], []
        tot = 0
        for s in range(NSUP):
            blks = []
            for c in range(s * NCH_S, (s + 1) * NCH_S):
                lst = []
                for (off, ln, wb) in segs[c]:
                    lst.append((off, ln, len(blks)))
                    blks.append(wb)
                dev_segs.append(lst)
            blocks.append(blks)
            kcount.append(len(blks))
            koff.append(tot)
            tot += len(blks)
        return dev_segs, blocks, kcount, koff, tot

    dseg1, blocks1, k1, off1, tot1 = pack_blocks(seg1)
    dseg2, blocks2, k2, off2, tot2 = pack_blocks(seg2)
    k1max, k2max = max(k1), max(k2)
    fallback = any(len(s) != 1 or s[0][1] != CH for s in dseg1 + dseg2)

    # ---- S^T matrices [128, N] bf16 ----------------------------------------
    j = np.arange(N)
    wtl = [(1 - fy[l]) * (1 - fx[l]) for l in range(3)]
    wtr = [(1 - fy[l]) * fx[l] for l in range(3)]
    wbl = [fy[l] * (1 - fx[l]) for l in range(3)]
    wbr = [fy[l] * fx[l] for l in range(3)]

    def wbase_arr(segs):
        wb = np.zeros(N, np.int64)
        for c in range(NCHUNK):
            for (off, ln, w) in segs[c]:
                wb[c * CH + off: c * CH + off + ln] = w
        return wb

    wb1 = wbase_arr(seg1)
    wb2 = wbase_arr(seg2)

    s_t = []
    for l, ktop, kbot in (
        (0, x0[0], 64 + x0[0]),
        (1, (y0[1] - wb1) * 32 + x0[1], (y0[1] - wb1 + 1) * 32 + x0[1]),
        (2, (y0[2] - wb2) * 16 + x0[2], (y0[2] - wb2 + 1) * 16 + x0[2]),
    ):
        Sm = np.zeros((N, 128), np.float32)
        Sm[j, ktop] = wtl[l]
        Sm[j, ktop + 1] = wtr[l]
        Sm[j, kbot] = wbl[l]
        Sm[j, kbot + 1] = wbr[l]
        s_t.append(np.ascontiguousarray(Sm.T).astype(BF16))
    # enc values folded into s2t rows 64:106
    s2t = s_t[2].astype(np.float32)
    s2t[64:106] = _posenc_t(coords_s)
    s_t[2] = s2t.astype(BF16)

    # ---- pyramid + folded A grids ------------------------------------------
    R1 = _resize_matrix(32, 64)
    R2 = _resize_matrix(16, 64)
    g1 = np.einsum("ph,qw,bhwc->bpqc", R1, R1, fg, optimize=True)
    g2 = np.einsum("ph,qw,bhwc->bpqc", R2, R2, fg, optimize=True)

    def fold(grid, wslice):  # [B, h, w, C] @ [C, 256] -> [B, h, w, 256]
        hh, ww = grid.shape[1], grid.shape[2]
        return (grid.reshape(B, -1, C) @ wslice).reshape(B, hh, ww, 256)

    A00 = fold(fg, w0[42:298]);  A30 = fold(fg, w3[298:554])
    A01 = fold(g1, w0[298:554]); A31 = fold(g1, w3[554:810])
    A02 = fold(g2, w0[554:810]); A32 = fold(g2, w3[810:1066])
    enc0 = w0[0:42]              # [42, 256]
    enc3 = w3[256:298]

    # ---- per-core stationary tensors ---------------------------------------
    def rp0_pack(Ab):  # [64, 64, 256] -> [128, 63*256] (bucket g: rows g,g+1)
        st = np.stack([Ab[:-1], Ab[1:]], axis=1)        # [63, 2, 64, 256]
        return np.ascontiguousarray(
            st.transpose(1, 2, 0, 3).reshape(128, 63 * 256)).astype(BF16)

    def rp1_pack(Ab, blks):  # [32, 32, 256] -> [128, k*256] 4-row windows
        Ap = np.zeros((32 + 3, 32, 256), np.float32)
        Ap[:32] = Ab
        cols = [Ap[w:w + 4].reshape(128, 256) for w in blks]
        return np.ascontiguousarray(np.concatenate(cols, axis=1)).astype(BF16)

    def rp2_pack(Ab, blks, wenc):  # [16,16,256]: 4-row window + enc rows
        Ap = np.zeros((16 + 3, 16, 256), np.float32)
        Ap[:16] = Ab
        cols = []
        for w in blks:
            blk = np.zeros((128, 256), np.float32)
            blk[0:64] = Ap[w:w + 4].reshape(64, 256)
            blk[64:106] = wenc
            cols.append(blk)
        return np.ascontiguousarray(np.concatenate(cols, axis=1)).astype(BF16)

    flat1 = [w for s in range(NSUP) for w in blocks1[s]]
    flat2 = [w for s in range(NSUP) for w in blocks2[s]]
    per_core = []
    for b in range(B):
        per_core.append({
            "rp0a": rp0_pack(A00[b]), "rp0b": rp0_pack(A30[b]),
            "rp1a": rp1_pack(A01[b], flat1), "rp1b": rp1_pack(A31[b], flat1),
            "rp2a": rp2_pack(A02[b], flat2, enc0),
            "rp2b": rp2_pack(A32[b], flat2, enc3),
        })

    # ---- MLP weights / biases ----------------------------------------------
    def pack(wd):  # [Ktot, M] -> [128, (Ktot/128) * M], k-tile major
        K, M = wd.shape
        assert K % 128 == 0
        return np.ascontiguousarray(
            wd.reshape(K // 128, 128, M).transpose(1, 0, 2).reshape(128, -1)
        )

    woutd = np.zeros((256, 3), np.float32)
    woutd[:] = w_out

    shared = {
        "s0t": s_t[0], "s1t": s_t[1], "s2t": s_t[2],
        "w1": pack(w1).astype(BF16), "w2": pack(w2).astype(BF16),
        "w3h": pack(w3[0:256]).astype(BF16),
        "wout": pack(woutd).astype(BF16),
        "b0": np.asarray(b0, np.float32).reshape(2, 128).T.copy(),
        "b1": np.asarray(b1, np.float32).reshape(2, 128).T.copy(),
        "b2": np.asarray(b2, np.float32).reshape(2, 128).T.copy(),
        "b3": np.asarray(b3, np.float32).reshape(2, 128).T.copy(),
        "bout": np.asarray(b_out, np.float32).reshape(3, 1).copy(),
        "wz": np.zeros((128, 128), BF16),
    }

    meta = {
        "runs0": runs0, "g_lo": g_lo, "g_wid": g_wid, "w0max": w0max,
        "dseg1": dseg1, "k1": k1, "off1": off1, "tot1": tot1, "k1max": k1max,
        "dseg2": dseg2, "k2": k2, "off2": off2, "tot2": tot2, "k2max": k2max,
        "fallback": fallback,
    }
    return shared, per_core, perm, meta


def _build_nc(meta):
    """Build the Bacc program (shared by all cores; per-core data differs)."""
    from contextlib import ExitStack

    import concourse.bacc as bacc
    import concourse.mybir as mybir
    import concourse.tile as tile

    bf16 = mybir.dt.bfloat16
    f32 = mybir.dt.float32
    GELU = mybir.ActivationFunctionType.Gelu_apprx_tanh
    TANH = mybir.ActivationFunctionType.Tanh

    runs0 = meta["runs0"]
    g_lo, g_wid, w0max = meta["g_lo"], meta["g_wid"], meta["w0max"]
    dseg1, k1, off1, tot1, k1max = (meta["dseg1"], meta["k1"], meta["off1"],
                                    meta["tot1"], meta["k1max"])
    dseg2, k2, off2, tot2, k2max = (meta["dseg2"], meta["k2"], meta["off2"],
                                    meta["tot2"], meta["k2max"])
    fallback = meta["fallback"]

    nc = bacc.Bacc("TRN2", debug=False, target_bir_lowering=False)

    specs = [
        ("s0t", (128, N), bf16), ("s1t", (128, N), bf16),
        ("s2t", (128, N), bf16),
        ("rp0a", (128, 63 * 256), bf16), ("rp0b", (128, 63 * 256), bf16),
        ("rp1a", (128, tot1 * 256), bf16), ("rp1b", (128, tot1 * 256), bf16),
        ("rp2a", (128, tot2 * 256), bf16), ("rp2b", (128, tot2 * 256), bf16),
        ("w1", (128, 512), bf16), ("w2", (128, 512), bf16),
        ("w3h", (128, 512), bf16), ("wout", (128, 6), bf16),
        ("b0", (128, 2), f32), ("b1", (128, 2), f32),
        ("b2", (128, 2), f32), ("b3", (128, 2), f32),
        ("bout", (3, 1), f32), ("wz", (128, 128), bf16),
    ]
    dram = {}
    for name, shape, dt in specs:
        dram[name] = nc.dram_tensor(name, list(shape), dt, kind="ExternalInput")
    out_dram = nc.dram_tensor("out_t", [3, N], f32, kind="ExternalOutput")

    with tile.TileContext(nc) as tc, ExitStack() as ctx:
        const = ctx.enter_context(tc.tile_pool(name="const", bufs=1))
        spool = ctx.enter_context(tc.tile_pool(name="stream", bufs=2))
        rpool = ctx.enter_context(tc.tile_pool(name="rp", bufs=2))
        hpool = ctx.enter_context(tc.tile_pool(name="h", bufs=2))
        opool = ctx.enter_context(tc.tile_pool(name="osb", bufs=2))
        ps = ctx.enter_context(tc.tile_pool(name="ps", bufs=4, space="PSUM"))

        # ---- static tensors -------------------------------------------------
        # a single dma_start already fans out over all 16 SDMA engines, but
        # each one costs ~1-2us serialized issue/completion on the HWDGE
        # ring.  So: few big DMAs, ordered by first-use time (the sampling
        # anchor needs s2t + r2a + nothing else).
        st = {}

        def load_const(*names):
            for name in names:
                shape, dt = next((s, d) for n, s, d in specs if n == name)
                t = const.tile(list(shape), dt, tag=name)
                nc.sync.dma_start(t[:, :], dram[name][:, :])
                st[name] = t

        if fallback:
            load_const("wz")

        for s in range(NSUP):
            lo = s * SUP
            sl = slice(lo, lo + SUP)

            def stream_tile(nm):
                t = spool.tile([128, SUP], bf16, tag=nm)
                nc.sync.dma_start(t[:, :], dram[nm][:, sl])
                return t

            def rp_slice(nm, width, doff, tag):
                t = rpool.tile([128, width * 256], bf16, tag=tag)
                nc.sync.dma_start(t[:, :0 + (doff[1] - doff[0]) * 256],
                                  dram[nm][:, doff[0] * 256: doff[1] * 256])
                return t

            s2 = stream_tile("s2t")
            r2a = rp_slice("rp2a", k2max, (off2[s], off2[s] + k2[s]), "r2a")
            if s == 0:
                load_const("b0")
            s0 = stream_tile("s0t")
            r0a = rp_slice("rp0a", w0max, (g_lo[s], g_lo[s] + g_wid[s]), "r0a")
            s1 = stream_tile("s1t")
            r1a = rp_slice("rp1a", k1max, (off1[s], off1[s] + k1[s]), "r1a")
            if s == 0:
                load_const("w1", "b1", "w2", "b2")
            r2b = rp_slice("rp2b", k2max, (off2[s], off2[s] + k2[s]), "r2b")
            r0b = rp_slice("rp0b", w0max, (g_lo[s], g_lo[s] + g_wid[s]), "r0b")
            r1b = rp_slice("rp1b", k1max, (off1[s], off1[s] + k1[s]), "r1b")
            if s == 0:
                load_const("w3h", "b3", "wout", "bout")
                bmlp = [st["b0"], st["b1"], st["b2"], st["b3"]]

            def sample_layer(P, m, hf, r2, r1, r0, extra=None):
                """Accumulate chunks (2*hf, 2*hf+1) of one m-half of the fused
                sampling+enc (+extra) pre-activation into PSUM tile P
                [128, 1024]."""
                for ch in range(2 * hf, 2 * hf + 2):
                    c0 = (ch - 2 * hf) * CH     # offset within P
                    g0 = ch * CH                # offset within super streams
                    psl = slice(c0, c0 + CH)
                    gsl = slice(g0, g0 + CH)
                    cidx = s * NCH_S + ch
                    # L2+enc: start anchor (full-width normal case)
                    sg2 = dseg2[cidx]
                    if len(sg2) == 1 and sg2[0][1] == CH:
                        off, ln, bi = sg2[0]
                        nc.tensor.matmul(
                            P[:, psl], r2[:, bi * 256 + m * 128:
                                          bi * 256 + m * 128 + 128],
                            s2[:, gsl], start=True, stop=False)
                    else:
                        nc.tensor.matmul(P[:, psl], st["wz"][:, :],
                                         s0[:, gsl], start=True, stop=False)
                        for (off, ln, bi) in sg2:
                            nc.tensor.matmul(
                                P[:, c0 + off: c0 + off + ln],
                                r2[:, bi * 256 + m * 128:
                                   bi * 256 + m * 128 + 128],
                                s2[:, g0 + off: g0 + off + ln],
                                start=False, stop=False)
                    if extra is not None:
                        extra(ch)
                    # L0 runs
                    for (g, off, ln) in runs0[cidx]:
                        nc.tensor.matmul(
                            P[:, c0 + off: c0 + off + ln],
                            r0[:, (g - g_lo[s]) * 256 + m * 128:
                               (g - g_lo[s]) * 256 + m * 128 + 128],
                            s0[:, g0 + off: g0 + off + ln],
                            start=False, stop=False)
                    # L1: stop anchor
                    sg1 = dseg1[cidx]
                    if len(sg1) == 1 and sg1[0][1] == CH:
                        off, ln, bi = sg1[0]
                        nc.tensor.matmul(
                            P[:, psl], r1[:, bi * 256 + m * 128:
                                          bi * 256 + m * 128 + 128],
                            s1[:, gsl], start=False, stop=True)
                    else:
                        for (off, ln, bi) in sg1:
                            nc.tensor.matmul(
                                P[:, c0 + off: c0 + off + ln],
                                r1[:, bi * 256 + m * 128:
                                   bi * 256 + m * 128 + 128],
                                s1[:, g0 + off: g0 + off + ln],
                                start=False, stop=False)
                        nc.tensor.matmul(P[:, psl], st["wz"][:, :],
                                         s0[:, gsl], start=False, stop=True)

            HF = 1024

            # ---- fused sampling + layer 0 ----------------------------------
            h0 = hpool.tile([128, 2 * SUP], bf16, tag="h0")
            for hf in range(2):
                for m in range(2):
                    P = ps.tile([128, HF], f32, tag="ps")
                    sample_layer(P, m, hf, r2a, r1a, r0a)
                    nc.scalar.activation(
                        h0[:, m * SUP + hf * HF: m * SUP + (hf + 1) * HF],
                        P[:, :], GELU, bias=bmlp[0][:, m:m + 1])

            # ---- layers 1, 2 -----------------------------------------------
            def dense(wt, rhs, bias, outtag):
                h = hpool.tile([128, 2 * SUP], bf16, tag=outtag)
                for hf in range(2):
                    for m in range(2):
                        P = ps.tile([128, HF], f32, tag="ps")
                        for ch in range(2 * hf, 2 * hf + 2):
                            psl = slice((ch - 2 * hf) * CH,
                                        (ch - 2 * hf + 1) * CH)
                            for kt in range(2):
                                nc.tensor.matmul(
                                    P[:, psl],
                                    wt[:, kt * 256 + m * 128:
                                       kt * 256 + m * 128 + 128],
                                    rhs[:, kt * SUP + ch * CH:
                                        kt * SUP + (ch + 1) * CH],
                                    start=(kt == 0), stop=(kt == 1))
                        nc.scalar.activation(
                            h[:, m * SUP + hf * HF: m * SUP + (hf + 1) * HF],
                            P[:, :], GELU, bias=bias[:, m:m + 1])
                return h

            h1 = dense(st["w1"], h0, bmlp[1], "h1")
            h2 = dense(st["w2"], h1, bmlp[2], "h2")

            # ---- layer 3 (skip concat folded) ------------------------------
            h3 = hpool.tile([128, 2 * SUP], bf16, tag="h3")
            for hf in range(2):
                for m in range(2):
                    P = ps.tile([128, HF], f32, tag="ps")

                    def extra(ch, P=P, m=m, hf=hf):
                        psl = slice((ch - 2 * hf) * CH, (ch - 2 * hf + 1) * CH)
                        for kt in range(2):
                            nc.tensor.matmul(
                                P[:, psl],
                                st["w3h"][:, kt * 256 + m * 128:
                                          kt * 256 + m * 128 + 128],
                                h2[:, kt * SUP + ch * CH:
                                   kt * SUP + (ch + 1) * CH],
                                start=False, stop=False)

                    sample_layer(P, m, hf, r2b, r1b, r0b, extra=extra)
                    nc.scalar.activation(
                        h3[:, m * SUP + hf * HF: m * SUP + (hf + 1) * HF],
                        P[:, :], GELU, bias=bmlp[3][:, m:m + 1])

            # ---- output layer ----------------------------------------------
            osb = opool.tile([3, SUP], f32, tag="osb")
            for hf in range(2):
                PO = ps.tile([128, HF], f32, tag="ps")
                for ch in range(2 * hf, 2 * hf + 2):
                    psl = slice((ch - 2 * hf) * CH, (ch - 2 * hf + 1) * CH)
                    for kt in range(2):
                        nc.tensor.matmul(
                            PO[:3, psl],
                            st["wout"][:, kt * 3:(kt + 1) * 3],
                            h3[:, kt * SUP + ch * CH: kt * SUP + (ch + 1) * CH],
                            start=(kt == 0), stop=(kt == 1))
                nc.scalar.activation(osb[:, hf * HF:(hf + 1) * HF],
                                     PO[:3, :], TANH, bias=st["bout"][:, 0:1])
            nc.sync.dma_start(out_dram[:, sl], osb[:, :])

    nc.compile()
    return nc


def kernel(feature_grid, coords, w0, b0, w1, b1, w2, b2, w3, b3, w_out, b_out,
           _run_opts=None):
    from concourse.bass_utils import run_bass_kernel_spmd

    shared, per_core, perm, meta = _host_prep(
        feature_grid, coords, w0, b0, w1, b1, w2, b2, w3, b3, w_out, b_out)

    nc = _build_nc(meta)

    in_maps = []
    for b in range(B):
        m = dict(shared)
        m.update(per_core[b])
        in_maps.append(m)

    res = run_bass_kernel_spmd(
        nc, in_maps, core_ids=list(range(B)), **(_run_opts or {})
    )

    out = np.empty((B, N, 3), np.float32)
    for b in range(B):
        out[b, perm, :] = res.results[b]["out_t"].T
    if _run_opts is not None:
        kernel._last_result = res  # for test harness introspection
    return out
